# revision 3
# baseline (speedup 1.0000x reference)
"""Trainium2 Bass kernel for GPLinear (geometric-product linear layer, Cl(3,0)).

    out[b,o,k] = sum_{i,j,p} G[i,j,k] * x[b,p,i] * W[p,o,j] + bias[o,k]

Algorithm: Cl(3,0) is isomorphic to the Pauli algebra M2(C).  Map the 8
blade coordinates of x and W to 2x2 complex matrices (each rep coordinate
is +-x_a +- x_b, done on host, exact in f32), then the geometric product
becomes complex 2x2 matrix multiplication:

    O[b,o,(r,c)] = sum_{p,s} X[b,p,(r,s)] * Wh[p,o,(s,c)]      (complex)

Realified (u/u' = re/im, u''=u^u', sign -1 iff u=u'=1) this is 32 real
[b x p] @ [p x o] matmuls -- HALF the 64 blade-pair matmuls the direct
XOR-sparse formulation needs.  Inverse transform (out_k = O_ga +- O_gb,
1/2 folded into Wh) is applied during PSUM evacuation.

Device mapping (per core, bf16 operands, f32 PSUM accumulation):
  - x-hat uploaded pre-transposed [p, a, b] with 12 coords (8 rep coords
    + 4 negated Im coords to realize the (-1)^(u u') sign via the
    stationary operand, since matmul can only accumulate).
  - w-hat uploaded [p, (s,c,u',o)]; bias uploaded in rep basis and
    pre-accumulated into PSUM via a K=1 ones-row matmul (start=True).
  - 96 matmuls of N=512/256, PSUM [b, (r,c,u'',o)] = 2048 f32 = 4 banks.
  - evacuation: ACT copies the r=1 half PSUM->SBUF (PSUM has one DVE
    read port), DVE tensor_tensor forms out_k = O_ga +- O_gb -> bf16.

Sharding (8 cores): 4-way batch x 2-way out_features.
core c -> batch rows [(c//2)*128, +128), out cols [(c%2)*256, +256).
"""

import numpy as np
import ml_dtypes

import concourse.bass as bass
import concourse.mybir as mybir
import concourse.tile as tile
from concourse import bacc
from concourse.bass_utils import run_bass_kernel_spmd

F32 = mybir.dt.float32
BF16 = mybir.dt.bfloat16
BF16_NP = ml_dtypes.bfloat16

BATCH, IN_F, OUT_F, K8 = 512, 512, 512, 8
R_B, R_O = 4, 2
N_CORES = R_B * R_O
B_LOC = BATCH // R_B            # 128 batch rows per core
OC = OUT_F // R_O               # 256 out features per core
PT = IN_F // 128                # 4 p-tiles

LAST_RESULTS = None


def _cayley_table() -> np.ndarray:
    G = np.zeros((8, 8, 8), dtype=np.float32)
    for a in range(8):
        for b in range(8):
            swaps, t = 0, a >> 1
            while t:
                swaps += bin(t & b).count("1")
                t >>= 1
            G[a, b, a ^ b] = -1.0 if (swaps & 1) else 1.0
    return G


def _check_G(G):
    assert np.array_equal(np.asarray(G, dtype=np.float32), _cayley_table()), \
        "G is not the Cl(3,0) Cayley table this kernel hardcodes"


# T8[alpha, i]: rep coord alpha=(r,s,u) (idx r*4+s*2+u) from blade i.
# Blades (bitmask): 0:1 1:e1 2:e2 3:e12 4:e3 5:e13 6:e23 7:e123.
def _T8():
    T = np.zeros((8, 8), dtype=np.float32)
    rows = {
        (0, 0, 0): [(0, 1), (4, 1)],
        (0, 0, 1): [(3, 1), (7, 1)],
        (0, 1, 0): [(1, 1), (5, -1)],
        (0, 1, 1): [(6, 1), (2, -1)],
        (1, 0, 0): [(1, 1), (5, 1)],
        (1, 0, 1): [(2, 1), (6, 1)],
        (1, 1, 0): [(0, 1), (4, -1)],
        (1, 1, 1): [(7, 1), (3, -1)],
    }
    for (r, s, u), terms in rows.items():
        for i, coef in terms:
            T[r * 4 + s * 2 + u, i] = coef
    return T


# S8[k, gamma]: out_k = sum_gamma S8[k,gamma] O_gamma (the 1/2 lives in wh)
def _S8():
    S = np.zeros((8, 8), dtype=np.float32)
    outrows = {
        0: [((0, 0, 0), 1), ((1, 1, 0), 1)],
        4: [((0, 0, 0), 1), ((1, 1, 0), -1)],
        3: [((0, 0, 1), 1), ((1, 1, 1), -1)],
        7: [((0, 0, 1), 1), ((1, 1, 1), 1)],
        1: [((0, 1, 0), 1), ((1, 0, 0), 1)],
        5: [((1, 0, 0), 1), ((0, 1, 0), -1)],
        2: [((1, 0, 1), 1), ((0, 1, 1), -1)],
        6: [((0, 1, 1), 1), ((1, 0, 1), 1)],
    }
    for k, terms in outrows.items():
        for (r, c, u), coef in terms:
            S[k, r * 4 + c * 2 + u] = coef
    return S


# evacuation pairs: k -> (col of O_(0,*) in r0 region, col of O_(1,*) in
# r1 region, op, r1_is_in0).  col within region = c*512 + u''*256.
_EVAC = {
    0: (0, 512, "add", False),
    4: (0, 512, "subtract", False),
    3: (256, 768, "subtract", False),
    7: (256, 768, "add", False),
    1: (512, 0, "add", False),
    5: (512, 0, "subtract", True),
    2: (768, 256, "subtract", True),
    6: (768, 256, "add", False),
}


def build_kernel(G, loop_n=None):
    _check_G(G)
    nc = bacc.Bacc("TRN2", target_bir_lowering=False, debug=False)

    xh_d = nc.dram_tensor("xh", [PT, 128, 12 * B_LOC], BF16, kind="ExternalInput")
    wh_d = nc.dram_tensor("wh", [PT, 128, 8 * OC], BF16, kind="ExternalInput")
    br_d = nc.dram_tensor("br", [1, 8 * OC], BF16, kind="ExternalInput")
    on_d = nc.dram_tensor("ones", [1, B_LOC], BF16, kind="ExternalInput")
    o_d = nc.dram_tensor("out", [B_LOC, OC * K8], BF16, kind="ExternalOutput")

    import contextlib

    with tile.TileContext(nc) as tc:
        with (
            tc.tile_pool(name="sb", bufs=1) as sb,
            tc.tile_pool(name="ps", bufs=1, space="PSUM") as ps,
            (tc.For_i(0, loop_n, 1) if loop_n else contextlib.nullcontext()),
        ):
            xh_sb = sb.tile([128, PT, 12 * B_LOC], BF16, tag="xh")
            wh_sb = sb.tile([128, PT, 8 * OC], BF16, tag="wh")
            br_sb = sb.tile([1, 8 * OC], BF16, tag="br")
            on_sb = sb.tile([1, B_LOC], BF16, tag="ones")
            r1_sb = sb.tile([128, 4 * OC], F32, tag="r1")
            out_sb = sb.tile([128, OC * K8], BF16, tag="out")
            og = ps.tile([128, 8 * OC], F32, tag="og")  # (r,c,u'',o) 4 banks

            # ---- DMA: small constants first, then per-p-tile chunks ----
            nc.sync.dma_start(on_sb[:, :], on_d.ap())
            nc.sync.dma_start(br_sb[:, :], br_d.ap())
            for t in range(PT):
                nc.sync.dma_start(xh_sb[:, t, :], xh_d.ap()[t])
                nc.sync.dma_start(wh_sb[:, t, :], wh_d.ap()[t])

            # ---- bias into PSUM: K=1 ones-row matmuls, start=True ----
            for r in range(2):
                for h in range(2):
                    off = r * 1024 + h * 512
                    nc.tensor.matmul(
                        og[:, off:off + 512],
                        on_sb[:, :],
                        br_sb[:, off:off + 512],
                        start=True, stop=False)

            # ---- 96 matmuls: O[(r,c,u''),o] += xh_a^T wh_b ----
            # a-coord SBUF index: (r,s,u) -> r*4+s*2+u; negated Im copies
            # at 8 + r*2 + s.
            for t in range(PT):
                for s in range(2):
                    for r in range(2):
                        a_re = (r * 4 + s * 2 + 0) * B_LOC
                        a_im = (r * 4 + s * 2 + 1) * B_LOC
                        a_imn = (8 + r * 2 + s) * B_LOC
                        # u=0 (Re X): N=512 chunks (u',o) for c=0,1; u''=u'
                        for c in range(2):
                            woff = s * 1024 + c * 512
                            ooff = r * 1024 + c * 512
                            nc.tensor.matmul(
                                og[:, ooff:ooff + 512],
                                xh_sb[:, t, a_re:a_re + B_LOC],
                                wh_sb[:, t, woff:woff + 512],
                                start=False, stop=False)
                        # u=1 (Im X): N=256 chunks; u'=0 -> u''=1 with +xh,
                        # u'=1 -> u''=0 with -xh.
                        for up, a_col in ((0, a_im), (1, a_imn)):
                            for c in range(2):
                                woff = s * 1024 + c * 512 + up * 256
                                ooff = r * 1024 + c * 512 + (1 - up) * 256
                                last = (t == PT - 1 and s == 1 and up == 1)
                                nc.tensor.matmul(
                                    og[:, ooff:ooff + 256],
                                    xh_sb[:, t, a_col:a_col + B_LOC],
                                    wh_sb[:, t, woff:woff + 256],
                                    start=False, stop=last)

            # ---- evacuation, split by o-half for store overlap ----
            ppitch = og[:].ap[0][0]
            spitch_r1 = r1_sb[:].ap[0][0]
            spitch_o = out_sb[:].ap[0][0]
            for oh in range(2):
                # ACT: r=1 region (4 chunks of 256, o-half each) -> SBUF f32
                src = bass.AP(tensor=og.tensor, offset=1024 + oh * 128,
                              ap=[[ppitch, 128], [256, 4], [1, 128]])
                dst = bass.AP(tensor=r1_sb.tensor, offset=oh * 128,
                              ap=[[spitch_r1, 128], [256, 4], [1, 128]])
                nc.scalar.copy(dst, src)
                # DVE: out_k = O_ga +- O_gb  (one op per blade k)
                for k in range(8):
                    c0, c1, opname, r1_first = _EVAC[k]
                    ap0 = bass.AP(tensor=og.tensor, offset=c0 + oh * 128,
                                  ap=[[ppitch, 128], [1, 128]])
                    ap1 = bass.AP(tensor=r1_sb.tensor, offset=c1 + oh * 128,
                                  ap=[[spitch_r1, 128], [1, 128]])
                    in0, in1 = (ap1, ap0) if r1_first else (ap0, ap1)
                    out_ap = bass.AP(tensor=out_sb.tensor,
                                     offset=oh * 1024 + k,
                                     ap=[[spitch_o, 128], [8, 128]])
                    nc.vector.tensor_tensor(
                        out=out_ap, in0=in0, in1=in1,
                        op=getattr(mybir.AluOpType, opname))
                nc.sync.dma_start(
                    o_d.ap()[:, oh * 1024:(oh + 1) * 1024],
                    out_sb[:, oh * 1024:(oh + 1) * 1024])

    nc.compile()
    return nc


def _host_transform(x, W, b):
    """Blade -> Pauli rep on host (exact +-1 f32 adds), bf16 cast, per-core
    slicing + pre-transposition into the device layouts."""
    x = np.asarray(x, dtype=np.float32)
    W = np.asarray(W, dtype=np.float32)
    b = np.asarray(b, dtype=np.float32)
    T8 = _T8()

    xh8 = np.einsum("bpi,ai->bpa", x, T8)            # [B, P, 8]
    # 12 coords: 8 + negated Im (a = r*4+s*2+1 for (r,s) row-major)
    xh12 = np.concatenate([xh8, -xh8[:, :, [1, 3, 5, 7]]], axis=2)
    xh12 = xh12.astype(BF16_NP)

    wh8 = np.einsum("poj,aj->poa", W, 0.5 * T8)      # [P, O, 8]
    wh8 = wh8.astype(BF16_NP)

    br8 = np.linalg.inv(_S8()).astype(np.float32) @ b.T   # [8 gamma, O]
    br8 = br8.astype(BF16_NP)

    ones = np.ones((1, B_LOC), dtype=BF16_NP)

    in_maps = []
    for c in range(N_CORES):
        bc, oc = divmod(c, R_O)
        xh_c = xh12[bc * B_LOC:(bc + 1) * B_LOC]          # [128, 512, 12]
        xh_c = np.ascontiguousarray(
            xh_c.transpose(1, 2, 0).reshape(PT, 128, 12 * B_LOC))
        wh_c = wh8[:, oc * OC:(oc + 1) * OC, :]           # [512, 256, 8]
        wh_c = np.ascontiguousarray(
            wh_c.transpose(0, 2, 1).reshape(PT, 128, 8 * OC))
        br_c = np.ascontiguousarray(
            br8[:, oc * OC:(oc + 1) * OC].reshape(1, 8 * OC))
        in_maps.append({"xh": xh_c, "wh": wh_c, "br": br_c, "ones": ones})
    return in_maps


def make_in_maps(x, W, b, G=None):
    return _host_transform(x, W, b)


_CACHE = {}


def kernel(x, W, b, G):
    global LAST_RESULTS
    _check_G(G)
    if "nc" not in _CACHE:
        _CACHE["nc"] = build_kernel(G)
    nc = _CACHE["nc"]

    in_maps = _host_transform(x, W, b)
    res = run_bass_kernel_spmd(nc, in_maps, core_ids=list(range(N_CORES)))
    LAST_RESULTS = res

    out = np.empty((BATCH, OUT_F, K8), dtype=np.float32)
    for c in range(N_CORES):
        bc, oc = divmod(c, R_O)
        o_c = np.asarray(res.results[c]["out"]).astype(np.float32)
        out[bc * B_LOC:(bc + 1) * B_LOC, oc * OC:(oc + 1) * OC, :] = \
            o_c.reshape(B_LOC, OC, K8)
    return out


# revision 6
# speedup vs baseline: 2.2522x; 2.2522x over previous
"""Trainium2 Bass kernel for GPLinear (geometric-product linear layer, Cl(3,0)).

    out[b,o,k] = sum_{i,j,p} G[i,j,k] * x[b,p,i] * W[p,o,j] + bias[o,k]

Algorithm: Cl(3,0) is isomorphic to the Pauli algebra M2(C).  Map the 8
blade coordinates of x and W to 2x2 complex matrices (each rep coordinate
is +-x_a +- x_b, done on host, exact in f32), then the geometric product
becomes complex 2x2 matrix multiplication:

    O[b,o,(r,c)] = sum_{p,s} X[b,p,(r,s)] * Wh[p,o,(s,c)]      (complex)

Realified (u/u' = re/im, u''=u^u', sign -1 iff u=u'=1) this is 32 real
[b x p] @ [p x o] matmuls -- HALF the 64 blade-pair matmuls the direct
XOR-sparse formulation needs.  The output stays in the rep basis on
device; the (2-sparse +-1) inverse transform and the bias add happen on
host, so evacuation is just PSUM->SBUF(bf16) copies.

Device mapping (per core, bf16 operands, f32 PSUM accumulation):
  - x-hat uploaded pre-transposed [p, (t,a,b)] with 12 coords (8 rep
    coords + 4 negated Im coords to realize the (-1)^(u u') sign via the
    stationary operand, since matmul can only accumulate).
  - w-hat uploaded [p, (t,s,c,u',o)].
  - 96 matmuls of N=512/256 into PSUM [b, (r,c,u'',o)] = 2048 f32.
  - DMA rings: x-hat on SP HWDGE, w-hat on ACT HWDGE, output stores on
    GPSIMD SWDGE -- three parallel queues, so steady-state loop
    iterations never head-of-line block.

Sharding (8 cores): 4-way batch x 2-way out_features.
core c -> batch rows [(c//2)*128, +128), out cols [(c%2)*256, +256).
"""

import numpy as np
import ml_dtypes

import concourse.bass as bass
import concourse.mybir as mybir
import concourse.tile as tile
from concourse import bacc
from concourse.bass_utils import run_bass_kernel_spmd

F32 = mybir.dt.float32
BF16 = mybir.dt.bfloat16
BF16_NP = ml_dtypes.bfloat16

BATCH, IN_F, OUT_F, K8 = 512, 512, 512, 8
R_B, R_O = 4, 2
N_CORES = R_B * R_O
B_LOC = BATCH // R_B            # 128 batch rows per core
OC = OUT_F // R_O               # 256 out features per core
PT = IN_F // 128                # 4 p-tiles

LAST_RESULTS = None


def _cayley_table() -> np.ndarray:
    G = np.zeros((8, 8, 8), dtype=np.float32)
    for a in range(8):
        for b in range(8):
            swaps, t = 0, a >> 1
            while t:
                swaps += bin(t & b).count("1")
                t >>= 1
            G[a, b, a ^ b] = -1.0 if (swaps & 1) else 1.0
    return G


def _check_G(G):
    assert np.array_equal(np.asarray(G, dtype=np.float32), _cayley_table()), \
        "G is not the Cl(3,0) Cayley table this kernel hardcodes"


# T8[alpha, i]: rep coord alpha=(r,s,u) (idx r*4+s*2+u) from blade i.
# Blades (bitmask): 0:1 1:e1 2:e2 3:e12 4:e3 5:e13 6:e23 7:e123.
def _T8():
    T = np.zeros((8, 8), dtype=np.float32)
    rows = {
        (0, 0, 0): [(0, 1), (4, 1)],
        (0, 0, 1): [(3, 1), (7, 1)],
        (0, 1, 0): [(1, 1), (5, -1)],
        (0, 1, 1): [(6, 1), (2, -1)],
        (1, 0, 0): [(1, 1), (5, 1)],
        (1, 0, 1): [(2, 1), (6, 1)],
        (1, 1, 0): [(0, 1), (4, -1)],
        (1, 1, 1): [(7, 1), (3, -1)],
    }
    for (r, s, u), terms in rows.items():
        for i, coef in terms:
            T[r * 4 + s * 2 + u, i] = coef
    return T


# S8[k, gamma]: out_k = sum_gamma S8[k,gamma] O_gamma (the 1/2 lives in wh)
def _S8():
    S = np.zeros((8, 8), dtype=np.float32)
    outrows = {
        0: [((0, 0, 0), 1), ((1, 1, 0), 1)],
        4: [((0, 0, 0), 1), ((1, 1, 0), -1)],
        3: [((0, 0, 1), 1), ((1, 1, 1), -1)],
        7: [((0, 0, 1), 1), ((1, 1, 1), 1)],
        1: [((0, 1, 0), 1), ((1, 0, 0), 1)],
        5: [((1, 0, 0), 1), ((0, 1, 0), -1)],
        2: [((1, 0, 1), 1), ((0, 1, 1), -1)],
        6: [((0, 1, 1), 1), ((1, 0, 1), 1)],
    }
    for k, terms in outrows.items():
        for (r, c, u), coef in terms:
            S[k, r * 4 + c * 2 + u] = coef
    return S


def build_kernel(G, loop_n=None):
    _check_G(G)
    nc = bacc.Bacc("TRN2", target_bir_lowering=False, debug=False)

    xh_d = nc.dram_tensor("xh", [128, PT * 8 * B_LOC], BF16,
                          kind="ExternalInput")
    wh_d = nc.dram_tensor("wh", [128, PT * 8 * OC], BF16,
                          kind="ExternalInput")
    o_d = nc.dram_tensor("out", [B_LOC, 8 * OC], BF16, kind="ExternalOutput")

    XT = 8 * B_LOC    # xh cols per p-tile
    WT = 8 * OC       # wh DMA cols per p-tile (v0, v1p per (s,c))
    WS = 12 * OC      # wh SBUF cols per p-tile (v0, v1p, v1n per (s,c))

    import contextlib

    with tile.TileContext(nc) as tc:
        with (
            tc.tile_pool(name="sb", bufs=1) as sb,
            tc.tile_pool(name="ps", bufs=1, space="PSUM") as ps,
            (tc.For_i(0, loop_n, 1) if loop_n else contextlib.nullcontext()),
        ):
            xh_sb = sb.tile([128, PT, XT], BF16, tag="xh")
            # per (t, s, c): [v0 | v1p | v1n] blocks of OC cols
            wh_sb = sb.tile([128, PT, WS], BF16, tag="wh")
            out_sb = sb.tile([128, 8 * OC], BF16, tag="out")
            og = ps.tile([128, 8 * OC], F32, tag="og")  # (r,c,u'',o) 4 banks

            pitch_wh = wh_sb[:].ap[0][0]

            # ---- DMA: xh chunks on SP ring, wh chunks on ACT ring.
            # wh DMA writes (v0, v1p) strided into the 3-block layout; DVE
            # then fills v1n = -v1p (sign of the Im x Im product).
            for t in range(PT):
                nc.sync.dma_start(
                    xh_sb[:, t, :], xh_d.ap()[:, t * XT:(t + 1) * XT])
                dst = bass.AP(tensor=wh_sb.tensor, offset=t * WS,
                              ap=[[pitch_wh, 128], [3 * OC, 4], [1, 2 * OC]])
                nc.scalar.dma_start(dst, wh_d.ap()[:, t * WT:(t + 1) * WT])
                v1p = bass.AP(tensor=wh_sb.tensor, offset=t * WS + OC,
                              ap=[[pitch_wh, 128], [3 * OC, 4], [1, OC]])
                v1n = bass.AP(tensor=wh_sb.tensor, offset=t * WS + 2 * OC,
                              ap=[[pitch_wh, 128], [3 * OC, 4], [1, OC]])
                nc.vector.tensor_scalar(
                    out=v1n, in0=v1p, scalar1=-1.0, scalar2=None,
                    op0=mybir.AluOpType.mult)

            # ---- 64 matmuls, all N=512: O[(r,c,u''),o] += xh_a^T wh_b ----
            # a-coord SBUF index: (r,s,u) -> r*4+s*2+u.
            # u=0 streams (v0, v1p) -> u''=(0,1); u=1 streams (v1n, v0) via
            # negative stride -> u''=(0,1) as well.
            for t in range(PT):
                for s in range(2):
                    for r in range(2):
                        a_re = (r * 4 + s * 2 + 0) * B_LOC
                        a_im = (r * 4 + s * 2 + 1) * B_LOC
                        first = (t == 0 and s == 0)
                        last = (t == PT - 1 and s == 1)
                        for u, a_col in ((0, a_re), (1, a_im)):
                            for c in range(2):
                                base = t * WS + (s * 2 + c) * 3 * OC
                                if u == 0:
                                    rhs = bass.AP(
                                        tensor=wh_sb.tensor, offset=base,
                                        ap=[[pitch_wh, 128], [1, 2 * OC]])
                                else:
                                    rhs = bass.AP(
                                        tensor=wh_sb.tensor,
                                        offset=base + 2 * OC,
                                        ap=[[pitch_wh, 128], [-2 * OC, 2],
                                            [1, OC]])
                                ooff = r * 1024 + c * 512
                                nc.tensor.matmul(
                                    og[:, ooff:ooff + 512],
                                    xh_sb[:, t, a_col:a_col + B_LOC],
                                    rhs,
                                    start=(first and u == 0),
                                    stop=(last and u == 1))

            # ---- evacuation: plain PSUM -> SBUF bf16 copies ----
            # r=0 half on DVE (its last MM retires before r=1's),
            # r=1 half on ACT; stores on the GPSIMD SWDGE ring.
            nc.vector.tensor_copy(out_sb[:, 0:1024], og[:, 0:1024])
            nc.gpsimd.dma_start(o_d.ap()[:, 0:1024], out_sb[:, 0:1024])
            nc.scalar.copy(out_sb[:, 1024:2048], og[:, 1024:2048])
            nc.gpsimd.dma_start(o_d.ap()[:, 1024:2048], out_sb[:, 1024:2048])

    nc.compile()
    return nc


def _host_transform(x, W, b=None):
    """Blade -> Pauli rep on host (exact +-1 f32 adds), bf16 cast, per-core
    slicing + pre-transposition into the device layouts."""
    x = np.asarray(x, dtype=np.float32)
    W = np.asarray(W, dtype=np.float32)
    T8 = _T8()

    xh8 = np.einsum("bpi,ai->bpa", x, T8).astype(BF16_NP)   # [B, P, 8]

    wh8 = np.einsum("poj,aj->poa", W, 0.5 * T8)      # [P, O, 8]
    wh8 = wh8.astype(BF16_NP)

    in_maps = []
    for c in range(N_CORES):
        bc, oc = divmod(c, R_O)
        xh_c = xh8[bc * B_LOC:(bc + 1) * B_LOC]           # [128, 512, 8]
        # [p_in_tile][t][a][b] -> rows p, cols (t, a, b)
        xh_c = np.ascontiguousarray(
            xh_c.transpose(1, 2, 0)                        # [512, 8, 128]
                .reshape(PT, 128, 8, B_LOC)                # [t, p, a, b]
                .transpose(1, 0, 2, 3)                     # [p, t, a, b]
                .reshape(128, PT * 8 * B_LOC))
        wh_c = wh8[:, oc * OC:(oc + 1) * OC, :]           # [512, 256, 8]
        wh_c = np.ascontiguousarray(
            wh_c.transpose(0, 2, 1)                        # [512, 8, 256]
                .reshape(PT, 128, 8, OC)                   # [t, p, beta, o]
                .transpose(1, 0, 2, 3)                     # [p, t, beta, o]
                .reshape(128, PT * 8 * OC))
        in_maps.append({"xh": xh_c, "wh": wh_c})
    return in_maps


def make_in_maps(x, W, b, G=None):
    return _host_transform(x, W, b)


_CACHE = {}


def kernel(x, W, b, G):
    global LAST_RESULTS
    _check_G(G)
    if "nc" not in _CACHE:
        _CACHE["nc"] = build_kernel(G)
    nc = _CACHE["nc"]

    in_maps = _host_transform(x, W)
    res = run_bass_kernel_spmd(nc, in_maps, core_ids=list(range(N_CORES)))
    LAST_RESULTS = res

    S8 = _S8()
    b = np.asarray(b, dtype=np.float32)
    out = np.empty((BATCH, OUT_F, K8), dtype=np.float32)
    for c in range(N_CORES):
        bc, oc = divmod(c, R_O)
        O = np.asarray(res.results[c]["out"]).astype(np.float32)
        O = O.reshape(B_LOC, 8, OC)                       # [b, gamma, o]
        o_c = np.einsum("kg,bgo->bok", S8, O) + b[oc * OC:(oc + 1) * OC]
        out[bc * B_LOC:(bc + 1) * B_LOC, oc * OC:(oc + 1) * OC, :] = o_c
    return out


# revision 11
# speedup vs baseline: 3.0547x; 1.3563x over previous
"""Trainium2 Bass kernel for GPLinear (geometric-product linear layer, Cl(3,0)).

    out[b,o,k] = sum_{i,j,p} G[i,j,k] * x[b,p,i] * W[p,o,j] + bias[o,k]

Algorithm: Cl(3,0) is isomorphic to the Pauli algebra M2(C).  Map the 8
blade coordinates of x and W to 2x2 complex matrices (each rep coordinate
is +-x_a +- x_b, done on host, exact in f32), then the geometric product
becomes complex 2x2 matrix multiplication:

    O[b,o,(r,c)] = sum_{p,s} X[b,p,(r,s)] * Wh[p,o,(s,c)]      (complex)

Realified (u/u' = re/im, u''=u^u', sign -1 iff u=u'=1) this is 32 real
[b x p] @ [p x o] matmuls -- HALF the 64 blade-pair matmuls the direct
XOR-sparse formulation needs.  The output stays in the rep basis on
device; the (2-sparse +-1) inverse transform and the bias add happen on
host, so evacuation is just PSUM->SBUF(bf16) copies.

Device mapping (per core, bf16 operands, f32 PSUM accumulation):
  - x-hat uploaded pre-transposed [p, (t,a,b)] with 12 coords (8 rep
    coords + 4 negated Im coords to realize the (-1)^(u u') sign via the
    stationary operand, since matmul can only accumulate).
  - w-hat uploaded [p, (t,s,c,u',o)].
  - 96 matmuls of N=512/256 into PSUM [b, (r,c,u'',o)] = 2048 f32.
  - DMA rings: x-hat on SP HWDGE, w-hat on ACT HWDGE, output stores on
    GPSIMD SWDGE -- three parallel queues, so steady-state loop
    iterations never head-of-line block.

Sharding (8 cores): 4-way batch x 2-way out_features.
core c -> batch rows [(c//2)*128, +128), out cols [(c%2)*256, +256).
"""

import numpy as np
import ml_dtypes

import concourse.bass as bass
import concourse.mybir as mybir
import concourse.tile as tile
from concourse import bacc
from concourse.bass_utils import run_bass_kernel_spmd

F32 = mybir.dt.float32
BF16 = mybir.dt.bfloat16
BF16_NP = ml_dtypes.bfloat16

BATCH, IN_F, OUT_F, K8 = 512, 512, 512, 8
R_B, R_O = 4, 2
N_CORES = R_B * R_O
B_LOC = BATCH // R_B            # 128 batch rows per core
OC = OUT_F // R_O               # 256 out features per core
PT = IN_F // 128                # 4 p-tiles

LAST_RESULTS = None


def _cayley_table() -> np.ndarray:
    G = np.zeros((8, 8, 8), dtype=np.float32)
    for a in range(8):
        for b in range(8):
            swaps, t = 0, a >> 1
            while t:
                swaps += bin(t & b).count("1")
                t >>= 1
            G[a, b, a ^ b] = -1.0 if (swaps & 1) else 1.0
    return G


def _check_G(G):
    assert np.array_equal(np.asarray(G, dtype=np.float32), _cayley_table()), \
        "G is not the Cl(3,0) Cayley table this kernel hardcodes"


# T8[alpha, i]: rep coord alpha=(r,s,u) (idx r*4+s*2+u) from blade i.
# Blades (bitmask): 0:1 1:e1 2:e2 3:e12 4:e3 5:e13 6:e23 7:e123.
def _T8():
    T = np.zeros((8, 8), dtype=np.float32)
    rows = {
        (0, 0, 0): [(0, 1), (4, 1)],
        (0, 0, 1): [(3, 1), (7, 1)],
        (0, 1, 0): [(1, 1), (5, -1)],
        (0, 1, 1): [(6, 1), (2, -1)],
        (1, 0, 0): [(1, 1), (5, 1)],
        (1, 0, 1): [(2, 1), (6, 1)],
        (1, 1, 0): [(0, 1), (4, -1)],
        (1, 1, 1): [(7, 1), (3, -1)],
    }
    for (r, s, u), terms in rows.items():
        for i, coef in terms:
            T[r * 4 + s * 2 + u, i] = coef
    return T


# S8[k, gamma]: out_k = sum_gamma S8[k,gamma] O_gamma (the 1/2 lives in wh)
def _S8():
    S = np.zeros((8, 8), dtype=np.float32)
    outrows = {
        0: [((0, 0, 0), 1), ((1, 1, 0), 1)],
        4: [((0, 0, 0), 1), ((1, 1, 0), -1)],
        3: [((0, 0, 1), 1), ((1, 1, 1), -1)],
        7: [((0, 0, 1), 1), ((1, 1, 1), 1)],
        1: [((0, 1, 0), 1), ((1, 0, 0), 1)],
        5: [((1, 0, 0), 1), ((0, 1, 0), -1)],
        2: [((1, 0, 1), 1), ((0, 1, 1), -1)],
        6: [((0, 1, 1), 1), ((1, 0, 1), 1)],
    }
    for k, terms in outrows.items():
        for (r, c, u), coef in terms:
            S[k, r * 4 + c * 2 + u] = coef
    return S


def build_kernel(G, loop_n=None, variant="full"):
    _check_G(G)
    nc = bacc.Bacc("TRN2", target_bir_lowering=False, debug=False)

    xh_d = nc.dram_tensor("xh", [128, PT * 8 * B_LOC], BF16,
                          kind="ExternalInput")
    wh_d = nc.dram_tensor("wh", [128, PT * 8 * OC], BF16,
                          kind="ExternalInput")
    o_d = nc.dram_tensor("out", [B_LOC, 8 * OC], BF16, kind="ExternalOutput")

    XT = 8 * B_LOC    # xh cols per p-tile
    WT = 8 * OC       # wh DMA cols per p-tile (v0, v1p per (s,c))
    WS = 12 * OC      # wh SBUF cols per p-tile (v0, v1p, v1n per (s,c))

    import contextlib

    with tile.TileContext(nc) as tc:
        with (
            tc.tile_pool(name="sb", bufs=1) as sb,
            tc.tile_pool(name="ps", bufs=1, space="PSUM") as ps,
        ):
            # Per-t tiles: WAR hazards stay per-p-tile, so next iteration's
            # DMA for tile t overlaps this iteration's matmuls on t' > t.
            xh_t = [sb.tile([128, XT], BF16, tag=f"xh{t}", name=f"xh{t}")
                    for t in range(PT)]
            wh_t = [sb.tile([128, WS], BF16, tag=f"wh{t}", name=f"wh{t}")
                    for t in range(PT)]
            out_sb = sb.tile([128, 8 * OC], BF16, tag="out")
            og = ps.tile([128, 8 * OC], F32, tag="og")  # (r,c,u'',o) 4 banks

            # wh block layout per (t, s, c): [v1n | v0 | v1p] blocks of OC:
            # u=0 streams [v0, v1p] (offset +OC), u=1 streams [v1n, v0]
            # (offset 0) -- both contiguous, both map linearly onto the
            # (u'', o) PSUM columns.
            def do_dma(negate=True):
                for t in range(PT):
                    pitch_wh = wh_t[t][:].ap[0][0]
                    nc.sync.dma_start(
                        xh_t[t][:], xh_d.ap()[:, t * XT:(t + 1) * XT])
                    dst = bass.AP(tensor=wh_t[t].tensor, offset=OC,
                                  ap=[[pitch_wh, 128], [3 * OC, 4],
                                      [1, 2 * OC]])
                    nc.scalar.dma_start(dst,
                                        wh_d.ap()[:, t * WT:(t + 1) * WT])
                    if not negate:
                        continue
                    v1p = bass.AP(tensor=wh_t[t].tensor, offset=2 * OC,
                                  ap=[[pitch_wh, 128], [3 * OC, 4], [1, OC]])
                    v1n = bass.AP(tensor=wh_t[t].tensor, offset=0,
                                  ap=[[pitch_wh, 128], [3 * OC, 4], [1, OC]])
                    nc.vector.tensor_scalar(
                        out=v1n, in0=v1p, scalar1=-1.0, scalar2=None,
                        op0=mybir.AluOpType.mult)

            # ---- 64 matmuls, all N=512 contiguous rhs ----
            # a-coord SBUF index: (r,s,u) -> r*4+s*2+u.  u=0 streams
            # [v0, v1p] (offset +OC within the (s,c) block), u=1 streams
            # [v1n, v0] (offset 0) -- both contiguous 512-col runs.
            def do_mms():
                for t in range(PT):
                    pitch_wh = wh_t[t][:].ap[0][0]
                    for s in range(2):
                        for r in range(2):
                            first = (t == 0 and s == 0)
                            last = (t == PT - 1 and s == 1)
                            for u in range(2):
                                a_col = (r * 4 + s * 2 + u) * B_LOC
                                for c in range(2):
                                    rhs = bass.AP(
                                        tensor=wh_t[t].tensor,
                                        offset=(s * 2 + c) * 3 * OC
                                               + (1 - u) * OC,
                                        ap=[[pitch_wh, 128], [1, 2 * OC]])
                                    ooff = r * 1024 + c * 512
                                    nc.tensor.matmul(
                                        og[:, ooff:ooff + 512],
                                        xh_t[t][:, a_col:a_col + B_LOC],
                                        rhs,
                                        start=(first and u == 0),
                                        stop=(last and u == 1))

            # ---- evacuation: plain PSUM -> SBUF bf16 copies ----
            # ACT takes r=0 (closes first), DVE takes r=1 (shorter op ->
            # smaller PSUM-WAR bubble before next iteration's first MM).
            def do_evac(store=True):
                nc.scalar.copy(out_sb[:, 0:1024], og[:, 0:1024])
                if store:
                    nc.gpsimd.dma_start(o_d.ap()[:, 0:1024],
                                        out_sb[:, 0:1024])
                nc.vector.tensor_copy(out_sb[:, 1024:2048], og[:, 1024:2048])
                if store:
                    nc.gpsimd.dma_start(o_d.ap()[:, 1024:2048],
                                        out_sb[:, 1024:2048])

            loop = (tc.For_i(0, loop_n, 1) if loop_n
                    else contextlib.nullcontext())
            if variant == "full":
                with loop:
                    do_dma()
                    do_mms()
                    do_evac()
            elif variant == "mm":
                do_dma()
                with loop:
                    do_mms()
                    do_evac(store=False)
            elif variant == "dma":
                with loop:
                    do_dma(negate=False)
            else:
                raise ValueError(variant)

    nc.compile()
    return nc


def _host_transform(x, W, b=None):
    """Blade -> Pauli rep on host (exact +-1 f32 adds), bf16 cast, per-core
    slicing + pre-transposition into the device layouts."""
    x = np.asarray(x, dtype=np.float32)
    W = np.asarray(W, dtype=np.float32)
    T8 = _T8()

    xh8 = np.einsum("bpi,ai->bpa", x, T8).astype(BF16_NP)   # [B, P, 8]

    wh8 = np.einsum("poj,aj->poa", W, 0.5 * T8)      # [P, O, 8]
    wh8 = wh8.astype(BF16_NP)

    in_maps = []
    for c in range(N_CORES):
        bc, oc = divmod(c, R_O)
        xh_c = xh8[bc * B_LOC:(bc + 1) * B_LOC]           # [128, 512, 8]
        # [p_in_tile][t][a][b] -> rows p, cols (t, a, b)
        xh_c = np.ascontiguousarray(
            xh_c.transpose(1, 2, 0)                        # [512, 8, 128]
                .reshape(PT, 128, 8, B_LOC)                # [t, p, a, b]
                .transpose(1, 0, 2, 3)                     # [p, t, a, b]
                .reshape(128, PT * 8 * B_LOC))
        wh_c = wh8[:, oc * OC:(oc + 1) * OC, :]           # [512, 256, 8]
        wh_c = np.ascontiguousarray(
            wh_c.transpose(0, 2, 1)                        # [512, 8, 256]
                .reshape(PT, 128, 8, OC)                   # [t, p, beta, o]
                .transpose(1, 0, 2, 3)                     # [p, t, beta, o]
                .reshape(128, PT * 8 * OC))
        in_maps.append({"xh": xh_c, "wh": wh_c})
    return in_maps


def make_in_maps(x, W, b, G=None):
    return _host_transform(x, W, b)


_CACHE = {}


def kernel(x, W, b, G):
    global LAST_RESULTS
    _check_G(G)
    if "nc" not in _CACHE:
        _CACHE["nc"] = build_kernel(G)
    nc = _CACHE["nc"]

    in_maps = _host_transform(x, W)
    res = run_bass_kernel_spmd(nc, in_maps, core_ids=list(range(N_CORES)))
    LAST_RESULTS = res

    S8 = _S8()
    b = np.asarray(b, dtype=np.float32)
    out = np.empty((BATCH, OUT_F, K8), dtype=np.float32)
    for c in range(N_CORES):
        bc, oc = divmod(c, R_O)
        O = np.asarray(res.results[c]["out"]).astype(np.float32)
        O = O.reshape(B_LOC, 8, OC)                       # [b, gamma, o]
        o_c = np.einsum("kg,bgo->bok", S8, O) + b[oc * OC:(oc + 1) * OC]
        out[bc * B_LOC:(bc + 1) * B_LOC, oc * OC:(oc + 1) * OC, :] = o_c
    return out


# revision 13
# speedup vs baseline: 3.2781x; 1.0732x over previous
"""Trainium2 Bass kernel for GPLinear (geometric-product linear layer, Cl(3,0)).

    out[b,o,k] = sum_{i,j,p} G[i,j,k] * x[b,p,i] * W[p,o,j] + bias[o,k]

Algorithm: Cl(3,0) is isomorphic to the Pauli algebra M2(C).  Map the 8
blade coordinates of x and W to 2x2 complex matrices (each rep coordinate
is +-x_a +- x_b, done on host, exact in f32), then the geometric product
becomes complex 2x2 matrix multiplication:

    O[b,o,(r,c)] = sum_{p,s} X[b,p,(r,s)] * Wh[p,o,(s,c)]      (complex)

Realified (u/u' = re/im, u''=u^u', sign -1 iff u=u'=1) this is 32 real
[b x p] @ [p x o] matmuls -- HALF the 64 blade-pair matmuls the direct
XOR-sparse formulation needs.  The output stays in the rep basis on
device; the (2-sparse +-1) inverse transform and the bias add happen on
host, so evacuation is just PSUM->SBUF(bf16) copies.

Device mapping (per core, bf16 operands, f32 PSUM accumulation):
  - x-hat uploaded pre-transposed [p, (t,a,b)] with 12 coords (8 rep
    coords + 4 negated Im coords to realize the (-1)^(u u') sign via the
    stationary operand, since matmul can only accumulate).
  - w-hat uploaded [p, (t,s,c,u',o)].
  - 96 matmuls of N=512/256 into PSUM [b, (r,c,u'',o)] = 2048 f32.
  - DMA rings: x-hat on SP HWDGE, w-hat on ACT HWDGE, output stores on
    GPSIMD SWDGE -- three parallel queues, so steady-state loop
    iterations never head-of-line block.

Sharding (8 cores): 4-way batch x 2-way out_features.
core c -> batch rows [(c//2)*128, +128), out cols [(c%2)*256, +256).
"""

import numpy as np
import ml_dtypes

import concourse.bass as bass
import concourse.mybir as mybir
import concourse.tile as tile
from concourse import bacc
from concourse.bass_utils import run_bass_kernel_spmd

F32 = mybir.dt.float32
BF16 = mybir.dt.bfloat16
BF16_NP = ml_dtypes.bfloat16

BATCH, IN_F, OUT_F, K8 = 512, 512, 512, 8
R_B, R_O = 4, 2
N_CORES = R_B * R_O
B_LOC = BATCH // R_B            # 128 batch rows per core
OC = OUT_F // R_O               # 256 out features per core
PT = IN_F // 128                # 4 p-tiles

LAST_RESULTS = None


def _cayley_table() -> np.ndarray:
    G = np.zeros((8, 8, 8), dtype=np.float32)
    for a in range(8):
        for b in range(8):
            swaps, t = 0, a >> 1
            while t:
                swaps += bin(t & b).count("1")
                t >>= 1
            G[a, b, a ^ b] = -1.0 if (swaps & 1) else 1.0
    return G


def _check_G(G):
    assert np.array_equal(np.asarray(G, dtype=np.float32), _cayley_table()), \
        "G is not the Cl(3,0) Cayley table this kernel hardcodes"


# T8[alpha, i]: rep coord alpha=(r,s,u) (idx r*4+s*2+u) from blade i.
# Blades (bitmask): 0:1 1:e1 2:e2 3:e12 4:e3 5:e13 6:e23 7:e123.
def _T8():
    T = np.zeros((8, 8), dtype=np.float32)
    rows = {
        (0, 0, 0): [(0, 1), (4, 1)],
        (0, 0, 1): [(3, 1), (7, 1)],
        (0, 1, 0): [(1, 1), (5, -1)],
        (0, 1, 1): [(6, 1), (2, -1)],
        (1, 0, 0): [(1, 1), (5, 1)],
        (1, 0, 1): [(2, 1), (6, 1)],
        (1, 1, 0): [(0, 1), (4, -1)],
        (1, 1, 1): [(7, 1), (3, -1)],
    }
    for (r, s, u), terms in rows.items():
        for i, coef in terms:
            T[r * 4 + s * 2 + u, i] = coef
    return T


# S8[k, gamma]: out_k = sum_gamma S8[k,gamma] O_gamma (the 1/2 lives in wh)
def _S8():
    S = np.zeros((8, 8), dtype=np.float32)
    outrows = {
        0: [((0, 0, 0), 1), ((1, 1, 0), 1)],
        4: [((0, 0, 0), 1), ((1, 1, 0), -1)],
        3: [((0, 0, 1), 1), ((1, 1, 1), -1)],
        7: [((0, 0, 1), 1), ((1, 1, 1), 1)],
        1: [((0, 1, 0), 1), ((1, 0, 0), 1)],
        5: [((1, 0, 0), 1), ((0, 1, 0), -1)],
        2: [((1, 0, 1), 1), ((0, 1, 1), -1)],
        6: [((0, 1, 1), 1), ((1, 0, 1), 1)],
    }
    for k, terms in outrows.items():
        for (r, c, u), coef in terms:
            S[k, r * 4 + c * 2 + u] = coef
    return S


def build_kernel(G, loop_n=None, variant="full"):
    _check_G(G)
    nc = bacc.Bacc("TRN2", target_bir_lowering=False, debug=False)

    xh_d = nc.dram_tensor("xh", [128, PT * 8 * B_LOC], BF16,
                          kind="ExternalInput")
    wh_d = nc.dram_tensor("wh", [128, PT * 8 * OC], BF16,
                          kind="ExternalInput")
    o_d = nc.dram_tensor("out", [B_LOC, 8 * OC], BF16, kind="ExternalOutput")

    XT = 8 * B_LOC    # xh cols per p-tile
    WT = 8 * OC       # wh DMA cols per p-tile (v0, v1p per (s,c))
    WS = 12 * OC      # wh SBUF cols per p-tile (v0, v1p, v1n per (s,c))

    import contextlib

    NB = 2  # double-buffer sets; loop body = NB logical iterations

    with tile.TileContext(nc) as tc:
        with (
            tc.tile_pool(name="sb", bufs=1) as sb,
            tc.tile_pool(name="ps", bufs=1, space="PSUM") as ps,
        ):
            # Two full buffer sets (SBUF tiles + PSUM): iteration i+1's
            # DMAs/matmuls never wait on iteration i's evacuation/stores.
            # Per-t tiles keep WAR hazards per-p-tile within a set.
            xh_t = [[sb.tile([128, XT], BF16, tag=f"xh{j}_{t}",
                             name=f"xh{j}_{t}") for t in range(PT)]
                    for j in range(NB)]
            wh_t = [[sb.tile([128, WS], BF16, tag=f"wh{j}_{t}",
                             name=f"wh{j}_{t}") for t in range(PT)]
                    for j in range(NB)]
            out_sb = [sb.tile([128, 8 * OC], BF16, tag=f"out{j}",
                              name=f"out{j}") for j in range(NB)]
            og = [ps.tile([128, 8 * OC], F32, tag=f"og{j}", name=f"og{j}")
                  for j in range(NB)]  # (r,c,u'',o) 4 banks each

            # wh block layout per (t, s, c): [v1n | v0 | v1p] blocks of OC:
            # u=0 streams [v0, v1p] (offset +OC), u=1 streams [v1n, v0]
            # (offset 0) -- both contiguous, both map linearly onto the
            # (u'', o) PSUM columns.
            def do_dma(j, negate=True):
                for t in range(PT):
                    w = wh_t[j][t]
                    pitch_wh = w[:].ap[0][0]
                    nc.sync.dma_start(
                        xh_t[j][t][:], xh_d.ap()[:, t * XT:(t + 1) * XT])
                    dst = bass.AP(tensor=w.tensor, offset=OC,
                                  ap=[[pitch_wh, 128], [3 * OC, 4],
                                      [1, 2 * OC]])
                    nc.sync.dma_start(dst,
                                      wh_d.ap()[:, t * WT:(t + 1) * WT])
                    if not negate:
                        continue
                    v1p = bass.AP(tensor=w.tensor, offset=2 * OC,
                                  ap=[[pitch_wh, 128], [3 * OC, 4], [1, OC]])
                    v1n = bass.AP(tensor=w.tensor, offset=0,
                                  ap=[[pitch_wh, 128], [3 * OC, 4], [1, OC]])
                    nc.vector.tensor_scalar(
                        out=v1n, in0=v1p, scalar1=-1.0, scalar2=None,
                        op0=mybir.AluOpType.mult)

            # ---- 64 matmuls, all N=512 contiguous rhs ----
            # a-coord SBUF index: (r,s,u) -> r*4+s*2+u.  u=0 streams
            # [v0, v1p] (offset +OC within the (s,c) block), u=1 streams
            # [v1n, v0] (offset 0) -- both contiguous 512-col runs.
            def do_mms(j):
                for t in range(PT):
                    w = wh_t[j][t]
                    pitch_wh = w[:].ap[0][0]
                    for s in range(2):
                        for r in range(2):
                            first = (t == 0 and s == 0)
                            last = (t == PT - 1 and s == 1)
                            for u in range(2):
                                a_col = (r * 4 + s * 2 + u) * B_LOC
                                for c in range(2):
                                    rhs = bass.AP(
                                        tensor=w.tensor,
                                        offset=(s * 2 + c) * 3 * OC
                                               + (1 - u) * OC,
                                        ap=[[pitch_wh, 128], [1, 2 * OC]])
                                    ooff = r * 1024 + c * 512
                                    nc.tensor.matmul(
                                        og[j][:, ooff:ooff + 512],
                                        xh_t[j][t][:, a_col:a_col + B_LOC],
                                        rhs,
                                        start=(first and u == 0),
                                        stop=(last and u == 1))

            # ---- evacuation: plain PSUM -> SBUF bf16 copies ----
            def do_evac(j, store=True):
                nc.scalar.copy(out_sb[j][:, 0:1024], og[j][:, 0:1024])
                if store:
                    nc.gpsimd.dma_start(o_d.ap()[:, 0:1024],
                                        out_sb[j][:, 0:1024])
                nc.vector.tensor_copy(out_sb[j][:, 1024:2048],
                                      og[j][:, 1024:2048])
                if store:
                    nc.gpsimd.dma_start(o_d.ap()[:, 1024:2048],
                                        out_sb[j][:, 1024:2048])

            def body(j, store=True):
                do_dma(j)
                do_mms(j)
                do_evac(j, store=store)

            if loop_n:
                assert loop_n % NB == 0, f"loop_n must be a multiple of {NB}"
            loop = (tc.For_i(0, loop_n // NB, 1) if loop_n
                    else contextlib.nullcontext())
            if variant == "full":
                if not loop_n:
                    body(0)   # single-shot: one logical execution
                else:
                    with loop:
                        for j in range(NB):
                            body(j)
            elif variant == "mm":
                for j in range(NB):
                    do_dma(j)
                with loop:
                    for j in range(NB):
                        do_mms(j)
                        do_evac(j, store=False)
            elif variant == "dma":
                with loop:
                    for j in range(NB):
                        do_dma(j, negate=False)
            else:
                raise ValueError(variant)

    nc.compile()
    return nc


def _host_transform(x, W, b=None):
    """Blade -> Pauli rep on host (exact +-1 f32 adds), bf16 cast, per-core
    slicing + pre-transposition into the device layouts."""
    x = np.asarray(x, dtype=np.float32)
    W = np.asarray(W, dtype=np.float32)
    T8 = _T8()

    xh8 = np.einsum("bpi,ai->bpa", x, T8).astype(BF16_NP)   # [B, P, 8]

    wh8 = np.einsum("poj,aj->poa", W, 0.5 * T8)      # [P, O, 8]
    wh8 = wh8.astype(BF16_NP)

    in_maps = []
    for c in range(N_CORES):
        bc, oc = divmod(c, R_O)
        xh_c = xh8[bc * B_LOC:(bc + 1) * B_LOC]           # [128, 512, 8]
        # [p_in_tile][t][a][b] -> rows p, cols (t, a, b)
        xh_c = np.ascontiguousarray(
            xh_c.transpose(1, 2, 0)                        # [512, 8, 128]
                .reshape(PT, 128, 8, B_LOC)                # [t, p, a, b]
                .transpose(1, 0, 2, 3)                     # [p, t, a, b]
                .reshape(128, PT * 8 * B_LOC))
        wh_c = wh8[:, oc * OC:(oc + 1) * OC, :]           # [512, 256, 8]
        wh_c = np.ascontiguousarray(
            wh_c.transpose(0, 2, 1)                        # [512, 8, 256]
                .reshape(PT, 128, 8, OC)                   # [t, p, beta, o]
                .transpose(1, 0, 2, 3)                     # [p, t, beta, o]
                .reshape(128, PT * 8 * OC))
        in_maps.append({"xh": xh_c, "wh": wh_c})
    return in_maps


def make_in_maps(x, W, b, G=None):
    return _host_transform(x, W, b)


_CACHE = {}


def kernel(x, W, b, G):
    global LAST_RESULTS
    _check_G(G)
    if "nc" not in _CACHE:
        _CACHE["nc"] = build_kernel(G)
    nc = _CACHE["nc"]

    in_maps = _host_transform(x, W)
    res = run_bass_kernel_spmd(nc, in_maps, core_ids=list(range(N_CORES)))
    LAST_RESULTS = res

    S8 = _S8()
    b = np.asarray(b, dtype=np.float32)
    out = np.empty((BATCH, OUT_F, K8), dtype=np.float32)
    for c in range(N_CORES):
        bc, oc = divmod(c, R_O)
        O = np.asarray(res.results[c]["out"]).astype(np.float32)
        O = O.reshape(B_LOC, 8, OC)                       # [b, gamma, o]
        o_c = np.einsum("kg,bgo->bok", S8, O) + b[oc * OC:(oc + 1) * OC]
        out[bc * B_LOC:(bc + 1) * B_LOC, oc * OC:(oc + 1) * OC, :] = o_c
    return out


# revision 14
# speedup vs baseline: 3.9930x; 1.2181x over previous
"""Trainium2 Bass kernel for GPLinear — Pauli rep + Karatsuba complex mult.

v3: on top of the M2(C) (Pauli) factorization (see v2), use the 3-mult
complex product:  m1 = aRe*bRe, m2 = aIm*bIm, m3 = (aRe+aIm)(bRe+bIm);
O_re = m1 - m2, O_im = m3 - m1 - m2.  The device accumulates the twelve
all-positive partial blocks M[v,(r,c)] = sum_{s,p} a(r,s,v) * w(s,c,v)
(v in {Re, Im, Sum}) — 48 matmuls of N=512, 25% fewer PE cycles than the
4-mult realification, and no sign handling on device at all.  The
m-recombination, inverse blade transform, and bias all happen on host.

Device layout (per core, bf16 operands, f32 PSUM):
  - xh: 8 coords uploaded (Re/Im per (r,s)); DVE forms the 4 Sum coords.
  - wh: 8 coords uploaded (Re/Im per (s,c)); DVE forms the 4 Sum coords.
  - PSUM: [b, (v, r, c, o)] = 3072 f32 = 6 banks.
  - DMA rings: xh on SP HWDGE, wh on ACT HWDGE, stores on GPSIMD SWDGE.

Sharding (8 cores): 4-way batch x 2-way out_features.
"""

import numpy as np
import ml_dtypes

import concourse.bass as bass
import concourse.mybir as mybir
import concourse.tile as tile
from concourse import bacc
from concourse.bass_utils import run_bass_kernel_spmd

F32 = mybir.dt.float32
BF16 = mybir.dt.bfloat16
BF16_NP = ml_dtypes.bfloat16

BATCH, IN_F, OUT_F, K8 = 512, 512, 512, 8
R_B, R_O = 4, 2
N_CORES = R_B * R_O
B_LOC = BATCH // R_B
OC = OUT_F // R_O
PT = IN_F // 128

LAST_RESULTS = None


def _cayley_table() -> np.ndarray:
    G = np.zeros((8, 8, 8), dtype=np.float32)
    for a in range(8):
        for b in range(8):
            swaps, t = 0, a >> 1
            while t:
                swaps += bin(t & b).count("1")
                t >>= 1
            G[a, b, a ^ b] = -1.0 if (swaps & 1) else 1.0
    return G


def _check_G(G):
    assert np.array_equal(np.asarray(G, dtype=np.float32), _cayley_table()), \
        "G is not the Cl(3,0) Cayley table this kernel hardcodes"


def _T8():
    T = np.zeros((8, 8), dtype=np.float32)
    rows = {
        (0, 0, 0): [(0, 1), (4, 1)],
        (0, 0, 1): [(3, 1), (7, 1)],
        (0, 1, 0): [(1, 1), (5, -1)],
        (0, 1, 1): [(6, 1), (2, -1)],
        (1, 0, 0): [(1, 1), (5, 1)],
        (1, 0, 1): [(2, 1), (6, 1)],
        (1, 1, 0): [(0, 1), (4, -1)],
        (1, 1, 1): [(7, 1), (3, -1)],
    }
    for (r, s, u), terms in rows.items():
        for i, coef in terms:
            T[r * 4 + s * 2 + u, i] = coef
    return T


def _S8():
    S = np.zeros((8, 8), dtype=np.float32)
    outrows = {
        0: [((0, 0, 0), 1), ((1, 1, 0), 1)],
        4: [((0, 0, 0), 1), ((1, 1, 0), -1)],
        3: [((0, 0, 1), 1), ((1, 1, 1), -1)],
        7: [((0, 0, 1), 1), ((1, 1, 1), 1)],
        1: [((0, 1, 0), 1), ((1, 0, 0), 1)],
        5: [((1, 0, 0), 1), ((0, 1, 0), -1)],
        2: [((1, 0, 1), 1), ((0, 1, 1), -1)],
        6: [((0, 1, 1), 1), ((1, 0, 1), 1)],
    }
    for k, terms in outrows.items():
        for (r, c, u), coef in terms:
            S[k, r * 4 + c * 2 + u] = coef
    return S


def build_kernel(G, loop_n=None, variant="full"):
    _check_G(G)
    nc = bacc.Bacc("TRN2", target_bir_lowering=False, debug=False)

    xh_d = nc.dram_tensor("xh", [128, PT * 8 * B_LOC], BF16,
                          kind="ExternalInput")
    wh_d = nc.dram_tensor("wh", [128, PT * 8 * OC], BF16,
                          kind="ExternalInput")
    o_d = nc.dram_tensor("out", [B_LOC, 12 * OC], BF16, kind="ExternalOutput")

    XD = 8 * B_LOC    # xh DMA cols per p-tile (Re, Im per (r,s))
    XS = 12 * B_LOC   # xh SBUF cols per p-tile (Re, Im, Sum per (r,s))
    WD = 8 * OC       # wh DMA cols per p-tile
    WS = 12 * OC      # wh SBUF cols per p-tile

    import contextlib

    NB = 2  # SBUF double-buffer sets; og (6 PSUM banks) is shared

    with tile.TileContext(nc) as tc:
        with (
            tc.tile_pool(name="sb", bufs=1) as sb,
            tc.tile_pool(name="ps", bufs=1, space="PSUM") as ps,
        ):
            # per (t, g): [Re | Im | Sum] blocks; g = (r,s) for xh, (s,c)
            # for wh.
            xh_t = [[sb.tile([128, XS], BF16, tag=f"xh{j}_{t}",
                             name=f"xh{j}_{t}") for t in range(PT)]
                    for j in range(NB)]
            wh_t = [[sb.tile([128, WS], BF16, tag=f"wh{j}_{t}",
                             name=f"wh{j}_{t}") for t in range(PT)]
                    for j in range(NB)]
            out_sb = [sb.tile([128, 12 * OC], BF16, tag=f"out{j}",
                              name=f"out{j}") for j in range(NB)]
            og = ps.tile([128, 12 * OC], F32, tag="og")  # (v,r,c,o) 6 banks

            def block3(tile_t, width, blk):
                pitch = tile_t[:].ap[0][0]
                return bass.AP(tensor=tile_t.tensor, offset=blk * width,
                               ap=[[pitch, 128], [3 * width, 4], [1, width]])

            def do_dma(j, sums=True):
                for t in range(PT):
                    x_, w_ = xh_t[j][t], wh_t[j][t]
                    pitch_xh = x_[:].ap[0][0]
                    pitch_wh = w_[:].ap[0][0]
                    dstx = bass.AP(tensor=x_.tensor, offset=0,
                                   ap=[[pitch_xh, 128], [3 * B_LOC, 4],
                                       [1, 2 * B_LOC]])
                    nc.sync.dma_start(dstx,
                                      xh_d.ap()[:, t * XD:(t + 1) * XD])
                    dstw = bass.AP(tensor=w_.tensor, offset=0,
                                   ap=[[pitch_wh, 128], [3 * OC, 4],
                                       [1, 2 * OC]])
                    nc.sync.dma_start(dstw,
                                      wh_d.ap()[:, t * WD:(t + 1) * WD])
                    if not sums:
                        continue
                    # DVE: Sum = Re + Im
                    nc.vector.tensor_tensor(
                        out=block3(x_, B_LOC, 2), in0=block3(x_, B_LOC, 0),
                        in1=block3(x_, B_LOC, 1), op=mybir.AluOpType.add)
                    nc.vector.tensor_tensor(
                        out=block3(w_, OC, 2), in0=block3(w_, OC, 0),
                        in1=block3(w_, OC, 1), op=mybir.AluOpType.add)

            # ---- 48 matmuls, all N=512: M[v,(r,c)] += a(r,s,v)^T w(s,c,v)
            def do_mms(j):
                for t in range(PT):
                    x_, w_ = xh_t[j][t], wh_t[j][t]
                    pitch_xh = x_[:].ap[0][0]
                    pitch_wh = w_[:].ap[0][0]
                    for s in range(2):
                        for r in range(2):
                            first = (t == 0 and s == 0)
                            last = (t == PT - 1 and s == 1)
                            for v in range(3):
                                a_col = ((r * 2 + s) * 3 + v) * B_LOC
                                rhs = bass.AP(
                                    tensor=w_.tensor,
                                    offset=s * 6 * OC + v * OC,
                                    ap=[[pitch_wh, 128], [3 * OC, 2],
                                        [1, OC]])
                                ooff = v * 1024 + r * 512
                                nc.tensor.matmul(
                                    og[:, ooff:ooff + 512],
                                    bass.AP(tensor=x_.tensor, offset=a_col,
                                            ap=[[pitch_xh, 128],
                                                [1, B_LOC]]),
                                    rhs,
                                    start=first, stop=last)

            # ---- evacuation: PSUM -> SBUF bf16, ACT early half + DVE late
            def do_evac(j, store=True):
                nc.scalar.copy(out_sb[j][:, 0:1536], og[:, 0:1536])
                if store:
                    nc.gpsimd.dma_start(o_d.ap()[:, 0:1536],
                                        out_sb[j][:, 0:1536])
                nc.vector.tensor_copy(out_sb[j][:, 1536:3072],
                                      og[:, 1536:3072])
                if store:
                    nc.gpsimd.dma_start(o_d.ap()[:, 1536:3072],
                                        out_sb[j][:, 1536:3072])

            def body(j, store=True):
                do_dma(j)
                do_mms(j)
                do_evac(j, store=store)

            if loop_n:
                assert loop_n % NB == 0, f"loop_n must be a multiple of {NB}"
            loop = (tc.For_i(0, loop_n // NB, 1) if loop_n
                    else contextlib.nullcontext())
            if variant == "full":
                if not loop_n:
                    body(0)
                else:
                    with loop:
                        for j in range(NB):
                            body(j)
            elif variant == "mm":
                for j in range(NB):
                    do_dma(j)
                with loop:
                    for j in range(NB):
                        do_mms(j)
                        do_evac(j, store=False)
            elif variant == "dma":
                with loop:
                    for j in range(NB):
                        do_dma(j, sums=False)
            else:
                raise ValueError(variant)

    nc.compile()
    return nc


def _host_transform(x, W, b=None):
    x = np.asarray(x, dtype=np.float32)
    W = np.asarray(W, dtype=np.float32)
    T8 = _T8()

    xh8 = np.einsum("bpi,ai->bpa", x, T8).astype(BF16_NP)   # [B,P,8] (r,s,u)
    wh8 = np.einsum("poj,aj->poa", W, 0.5 * T8).astype(BF16_NP)  # (s,c,u')

    in_maps = []
    for c in range(N_CORES):
        bc, oc = divmod(c, R_O)
        xh_c = xh8[bc * B_LOC:(bc + 1) * B_LOC]           # [128, 512, 8]
        xh_c = np.ascontiguousarray(
            xh_c.transpose(1, 2, 0)                        # [512, 8, 128]
                .reshape(PT, 128, 8, B_LOC)                # [t, p, (r,s,u), b]
                .transpose(1, 0, 2, 3)
                .reshape(128, PT * 8 * B_LOC))
        wh_c = wh8[:, oc * OC:(oc + 1) * OC, :]           # [512, 256, 8]
        wh_c = np.ascontiguousarray(
            wh_c.transpose(0, 2, 1)                        # [512, 8, 256]
                .reshape(PT, 128, 8, OC)                   # [t, p, (s,c,u'), o]
                .transpose(1, 0, 2, 3)
                .reshape(128, PT * 8 * OC))
        in_maps.append({"xh": xh_c, "wh": wh_c})
    return in_maps


def make_in_maps(x, W, b, G=None):
    return _host_transform(x, W, b)


_CACHE = {}


def kernel(x, W, b, G):
    global LAST_RESULTS
    _check_G(G)
    if "nc" not in _CACHE:
        _CACHE["nc"] = build_kernel(G)
    nc = _CACHE["nc"]

    in_maps = _host_transform(x, W)
    res = run_bass_kernel_spmd(nc, in_maps, core_ids=list(range(N_CORES)))
    LAST_RESULTS = res

    S8 = _S8()
    b = np.asarray(b, dtype=np.float32)
    out = np.empty((BATCH, OUT_F, K8), dtype=np.float32)
    for c in range(N_CORES):
        bc, oc = divmod(c, R_O)
        M = np.asarray(res.results[c]["out"]).astype(np.float32)
        M = M.reshape(B_LOC, 3, 2, 2, OC)                 # [b, v, r, c, o]
        O = np.empty((B_LOC, 2, 2, 2, OC), dtype=np.float32)  # [b,r,c,u'',o]
        O[:, :, :, 0] = M[:, 0] - M[:, 1]                 # m1 - m2
        O[:, :, :, 1] = M[:, 2] - M[:, 0] - M[:, 1]       # m3 - m1 - m2
        O = O.reshape(B_LOC, 8, OC)                       # gamma = (r,c,u'')
        o_c = np.einsum("kg,bgo->bok", S8, O) + b[oc * OC:(oc + 1) * OC]
        out[bc * B_LOC:(bc + 1) * B_LOC, oc * OC:(oc + 1) * OC, :] = o_c
    return out


# revision 17
# speedup vs baseline: 4.4334x; 1.1103x over previous
"""Trainium2 Bass kernel for GPLinear — Pauli rep + Karatsuba complex mult.

v3: on top of the M2(C) (Pauli) factorization (see v2), use the 3-mult
complex product:  m1 = aRe*bRe, m2 = aIm*bIm, m3 = (aRe+aIm)(bRe+bIm);
O_re = m1 - m2, O_im = m3 - m1 - m2.  The device accumulates the twelve
all-positive partial blocks M[v,(r,c)] = sum_{s,p} a(r,s,v) * w(s,c,v)
(v in {Re, Im, Sum}) — 48 matmuls of N=512, 25% fewer PE cycles than the
4-mult realification, and no sign handling on device at all.  The
m-recombination, inverse blade transform, and bias all happen on host.

Device layout (per core, bf16 operands, f32 PSUM):
  - xh: 8 coords uploaded (Re/Im per (r,s)); DVE forms the 4 Sum coords.
  - wh: 8 coords uploaded (Re/Im per (s,c)); DVE forms the 4 Sum coords.
  - PSUM: [b, (v, r, c, o)] = 3072 f32 = 6 banks.
  - DMA rings: xh on SP HWDGE, wh on ACT HWDGE, stores on GPSIMD SWDGE.

Sharding (8 cores): 4-way batch x 2-way out_features.
"""

import numpy as np
import ml_dtypes

import concourse.bass as bass
import concourse.mybir as mybir
import concourse.tile as tile
from concourse import bacc
from concourse.bass_utils import run_bass_kernel_spmd

F32 = mybir.dt.float32
BF16 = mybir.dt.bfloat16
BF16_NP = ml_dtypes.bfloat16

BATCH, IN_F, OUT_F, K8 = 512, 512, 512, 8
R_B, R_O = 4, 2
N_CORES = R_B * R_O
B_LOC = BATCH // R_B
OC = OUT_F // R_O
PT = IN_F // 128

LAST_RESULTS = None


def _cayley_table() -> np.ndarray:
    G = np.zeros((8, 8, 8), dtype=np.float32)
    for a in range(8):
        for b in range(8):
            swaps, t = 0, a >> 1
            while t:
                swaps += bin(t & b).count("1")
                t >>= 1
            G[a, b, a ^ b] = -1.0 if (swaps & 1) else 1.0
    return G


def _check_G(G):
    assert np.array_equal(np.asarray(G, dtype=np.float32), _cayley_table()), \
        "G is not the Cl(3,0) Cayley table this kernel hardcodes"


def _T8():
    T = np.zeros((8, 8), dtype=np.float32)
    rows = {
        (0, 0, 0): [(0, 1), (4, 1)],
        (0, 0, 1): [(3, 1), (7, 1)],
        (0, 1, 0): [(1, 1), (5, -1)],
        (0, 1, 1): [(6, 1), (2, -1)],
        (1, 0, 0): [(1, 1), (5, 1)],
        (1, 0, 1): [(2, 1), (6, 1)],
        (1, 1, 0): [(0, 1), (4, -1)],
        (1, 1, 1): [(7, 1), (3, -1)],
    }
    for (r, s, u), terms in rows.items():
        for i, coef in terms:
            T[r * 4 + s * 2 + u, i] = coef
    return T


def _S8():
    S = np.zeros((8, 8), dtype=np.float32)
    outrows = {
        0: [((0, 0, 0), 1), ((1, 1, 0), 1)],
        4: [((0, 0, 0), 1), ((1, 1, 0), -1)],
        3: [((0, 0, 1), 1), ((1, 1, 1), -1)],
        7: [((0, 0, 1), 1), ((1, 1, 1), 1)],
        1: [((0, 1, 0), 1), ((1, 0, 0), 1)],
        5: [((1, 0, 0), 1), ((0, 1, 0), -1)],
        2: [((1, 0, 1), 1), ((0, 1, 1), -1)],
        6: [((0, 1, 1), 1), ((1, 0, 1), 1)],
    }
    for k, terms in outrows.items():
        for (r, c, u), coef in terms:
            S[k, r * 4 + c * 2 + u] = coef
    return S


def build_kernel(G, loop_n=None, variant="full"):
    _check_G(G)
    nc = bacc.Bacc("TRN2", target_bir_lowering=False, debug=False)

    xh_d = nc.dram_tensor("xh", [128, PT * 12 * B_LOC], BF16,
                          kind="ExternalInput")
    wh_d = nc.dram_tensor("wh", [128, PT * 12 * OC], BF16,
                          kind="ExternalInput")
    o_d = nc.dram_tensor("out", [B_LOC, 12 * OC], BF16, kind="ExternalOutput")

    XS = 12 * B_LOC   # xh cols per p-tile (Re, Im, Sum per (r,s))
    WS = 12 * OC      # wh cols per p-tile (Re, Im, Sum per (s,c))

    import contextlib

    NB = 2  # SBUF double-buffer sets; og (6 PSUM banks) is shared

    with tile.TileContext(nc) as tc:
        with (
            tc.tile_pool(name="sb", bufs=1) as sb,
            tc.tile_pool(name="ps", bufs=1, space="PSUM") as ps,
        ):
            # per (t, g): [Re | Im | Sum] blocks; g = (r,s) for xh, (s,c)
            # for wh.
            xh_t = [[sb.tile([128, XS], BF16, tag=f"xh{j}_{t}",
                             name=f"xh{j}_{t}") for t in range(PT)]
                    for j in range(NB)]
            wh_t = [[sb.tile([128, WS], BF16, tag=f"wh{j}_{t}",
                             name=f"wh{j}_{t}") for t in range(PT)]
                    for j in range(NB)]
            out_sb = [sb.tile([128, 12 * OC], BF16, tag=f"out{j}",
                              name=f"out{j}") for j in range(NB)]
            og = ps.tile([128, 12 * OC], F32, tag="og")  # (v,r,c,o) 6 banks

            def do_dma(j, sums=True):
                # contiguous uploads: wh (3MB) on the SP HWDGE ring,
                # xh (1.5MB) on the ACT HWDGE ring
                for t in range(PT):
                    nc.scalar.dma_start(
                        xh_t[j][t][:], xh_d.ap()[:, t * XS:(t + 1) * XS])
                    nc.sync.dma_start(
                        wh_t[j][t][:], wh_d.ap()[:, t * WS:(t + 1) * WS])

            # ---- 48 matmuls, all N=512: M[v,(r,c)] += a(r,s,v)^T w(s,c,v)
            def do_mms(j):
                for t in range(PT):
                    x_, w_ = xh_t[j][t], wh_t[j][t]
                    pitch_xh = x_[:].ap[0][0]
                    pitch_wh = w_[:].ap[0][0]
                    for s in range(2):
                        for r in range(2):
                            first = (t == 0 and s == 0)
                            last = (t == PT - 1 and s == 1)
                            for v in range(3):
                                a_col = ((r * 2 + s) * 3 + v) * B_LOC
                                rhs = bass.AP(
                                    tensor=w_.tensor,
                                    offset=s * 6 * OC + v * OC,
                                    ap=[[pitch_wh, 128], [3 * OC, 2],
                                        [1, OC]])
                                ooff = v * 1024 + r * 512
                                nc.tensor.matmul(
                                    og[:, ooff:ooff + 512],
                                    bass.AP(tensor=x_.tensor, offset=a_col,
                                            ap=[[pitch_xh, 128],
                                                [1, B_LOC]]),
                                    rhs,
                                    start=first, stop=last)

            # ---- evacuation: PSUM -> SBUF bf16, ACT early half + DVE late
            def do_evac(j, store=True):
                nc.scalar.copy(out_sb[j][:, 0:1536], og[:, 0:1536])
                if store:
                    nc.gpsimd.dma_start(o_d.ap()[:, 0:1536],
                                        out_sb[j][:, 0:1536])
                nc.vector.tensor_copy(out_sb[j][:, 1536:3072],
                                      og[:, 1536:3072])
                if store:
                    nc.gpsimd.dma_start(o_d.ap()[:, 1536:3072],
                                        out_sb[j][:, 1536:3072])

            def body(j, store=True):
                do_dma(j)
                do_mms(j)
                do_evac(j, store=store)

            if loop_n:
                assert loop_n % NB == 0, f"loop_n must be a multiple of {NB}"
            loop = (tc.For_i(0, loop_n // NB, 1) if loop_n
                    else contextlib.nullcontext())
            if variant == "full":
                if not loop_n:
                    body(0)
                else:
                    with loop:
                        for j in range(NB):
                            body(j)
            elif variant == "mm":
                for j in range(NB):
                    do_dma(j)
                with loop:
                    for j in range(NB):
                        do_mms(j)
                        do_evac(j, store=False)
            elif variant == "dma":
                with loop:
                    for j in range(NB):
                        do_dma(j, sums=False)
            else:
                raise ValueError(variant)

    nc.compile()
    return nc


def _host_transform(x, W, b=None):
    x = np.asarray(x, dtype=np.float32)
    W = np.asarray(W, dtype=np.float32)
    T8 = _T8()

    def to12(a8):
        # [..., (g,u)] -> [..., (g, {Re, Im, Re+Im})], exact f32 adds
        g = a8.reshape(*a8.shape[:-1], 4, 2)
        return np.concatenate([g, g.sum(axis=-1, keepdims=True)],
                              axis=-1).reshape(*a8.shape[:-1], 12)

    xh12 = to12(np.einsum("bpi,ai->bpa", x, T8)).astype(BF16_NP)
    wh12 = to12(np.einsum("poj,aj->poa", W, 0.5 * T8)).astype(BF16_NP)

    in_maps = []
    for c in range(N_CORES):
        bc, oc = divmod(c, R_O)
        xh_c = xh12[bc * B_LOC:(bc + 1) * B_LOC]          # [128, 512, 12]
        xh_c = np.ascontiguousarray(
            xh_c.transpose(1, 2, 0)                        # [512, 12, 128]
                .reshape(PT, 128, 12, B_LOC)               # [t, p, (g,v), b]
                .transpose(1, 0, 2, 3)
                .reshape(128, PT * 12 * B_LOC))
        wh_c = wh12[:, oc * OC:(oc + 1) * OC, :]          # [512, 256, 12]
        wh_c = np.ascontiguousarray(
            wh_c.transpose(0, 2, 1)                        # [512, 12, 256]
                .reshape(PT, 128, 12, OC)                  # [t, p, (g,v), o]
                .transpose(1, 0, 2, 3)
                .reshape(128, PT * 12 * OC))
        in_maps.append({"xh": xh_c, "wh": wh_c})
    return in_maps


def make_in_maps(x, W, b, G=None):
    return _host_transform(x, W, b)


_CACHE = {}


def kernel(x, W, b, G):
    global LAST_RESULTS
    _check_G(G)
    if "nc" not in _CACHE:
        _CACHE["nc"] = build_kernel(G)
    nc = _CACHE["nc"]

    in_maps = _host_transform(x, W)
    res = run_bass_kernel_spmd(nc, in_maps, core_ids=list(range(N_CORES)))
    LAST_RESULTS = res

    S8 = _S8()
    b = np.asarray(b, dtype=np.float32)
    out = np.empty((BATCH, OUT_F, K8), dtype=np.float32)
    for c in range(N_CORES):
        bc, oc = divmod(c, R_O)
        M = np.asarray(res.results[c]["out"]).astype(np.float32)
        M = M.reshape(B_LOC, 3, 2, 2, OC)                 # [b, v, r, c, o]
        O = np.empty((B_LOC, 2, 2, 2, OC), dtype=np.float32)  # [b,r,c,u'',o]
        O[:, :, :, 0] = M[:, 0] - M[:, 1]                 # m1 - m2
        O[:, :, :, 1] = M[:, 2] - M[:, 0] - M[:, 1]       # m3 - m1 - m2
        O = O.reshape(B_LOC, 8, OC)                       # gamma = (r,c,u'')
        o_c = np.einsum("kg,bgo->bok", S8, O) + b[oc * OC:(oc + 1) * OC]
        out[bc * B_LOC:(bc + 1) * B_LOC, oc * OC:(oc + 1) * OC, :] = o_c
    return out
